# revision 2
# baseline (speedup 1.0000x reference)
"""Trainium2 Bass kernel for IR-Net style binarized 3x3 conv + BN + Hardtanh.

Reference computation:
  bw = sign(standardize(weight)) * sw   (sw = per-cout power-of-2 scale)
  ba = sign(x)
  y  = clip(conv3x3(ba, bw) * bn_scale + bn_bias, -1, 1)

Both matmul operands are exactly +-1, which is exactly representable in
fp8e4m3, so the conv runs as fp8 DoubleRow matmuls on the TensorEngine
with zero numerical error (fp32 PSUM accumulation of integers <= 2304).
Weight standardization/sign, sw, and BN folding are host-side prep
(0.6 MB of data); sw and bn scale fold into a single per-channel scale
applied in the epilogue (on VectorE, so ScalarE is free for binarize).

Distribution: pure data parallel, 32 images -> 4 per NeuronCore, full
weights replicated, no collectives.

Layout: per-image zero-padded 58x58 activation planes in SBUF, fp8, with
the two cin-128-chunks stacked as the DoubleRow k-subtile dim.  Each of
the 9 conv taps is then a contiguous shifted window of the flattened
padded plane, so the conv becomes 9 accumulated DoubleRow matmuls
([128,2,128] @ [128,2,464], K=256) per 8-row output tile.

v2 vs v1 (121.5us): the warm matmul stream already runs at the exact
464-cycle/2.4GHz floor, so all remaining time is edge overhead.
 - All input DMAs go through the sync queue in explicit FIFO order
   (one shared HW DMA engine serves all dynamic queues): img0 chunk0
   first, then the weights split into 9 per-tap DMAs, so the first
   matmul's operands land ~4us earlier than with the monolithic 1.2MB
   weight DMA in front.
 - 10 warm-up matmuls on garbage SBUF data issue as the first Tensor
   instructions: the PE is idle during the DMA ramp anyway, and ~3.9us
   of sustained PE activity flips the HAM clock gate to 8/8 before the
   real stream begins (v1 paid 14 matmuls at the 1.2GHz cold clock).
 - Output is stored as bf16 (exact for the clipped +-1 majority, 2^-9
   relative on the rest) halving output DMA and the end-of-kernel DMA
   drain; host casts back to fp32.
"""

import numpy as np

import concourse.bass as bass
import concourse.bacc as bacc
import concourse.mybir as mybir
import concourse.tile as tile
from concourse.bass_utils import run_bass_kernel_spmd

B, CIN, COUT, H, W = 32, 256, 256, 56, 56
NCORES = 8
BPC = B // NCORES            # images per core
HP, WP = H + 2, W + 2        # zero-padded plane
IMG = HP * WP                # 3364
GUARD = 64                   # front zero guard (shifted windows stay in bounds)
XT = 3504                    # GUARD + IMG + tail guard(76); %16==0 for DoubleRow
RB = 8                       # output rows per tile
NBLK = H // RB               # 7
NT = RB * WP                 # 464 matmul free dim (incl. 2 garbage cols/row)
NCI = CIN // 128             # 2 cin chunks = DoubleRow k-subtiles
NCO = COUT // 128            # 2 cout chunks
KTAPS = 9
NWARM = 10                   # HAM warm-up matmuls (~3.9us cold PE activity)
BN_EPS = 1e-5

F32 = mybir.dt.float32
FP8 = mybir.dt.float8e4
BF16 = mybir.dt.bfloat16

_CACHE: dict = {}


def _build_nc() -> bass.Bass:
    nc = bacc.Bacc("TRN2", target_bir_lowering=False, debug=False, num_devices=NCORES)
    xin = nc.declare_dram_parameter("xin", [BPC, CIN, H * W], BF16, isOutput=False)
    wts = nc.declare_dram_parameter(
        "wts", [128, KTAPS * NCO * NCI * 128], FP8, isOutput=False
    )
    sb = nc.declare_dram_parameter("sb", [128, 2 * NCO], F32, isOutput=False)
    yout = nc.declare_dram_parameter("yout", [BPC, COUT, H, W], BF16, isOutput=True)

    # img0 binarize chunk row counts: first chunk is exactly what output
    # block 0 needs (x rows 0..8), so the first matmul starts ASAP.
    CHUNKS = [10, 16, 15, 15]
    assert sum(CHUNKS) == H
    WCOL = NCO * NCI * 128    # weight columns per tap

    with tile.TileContext(nc) as tc:
        with (
            tc.tile_pool(name="const", bufs=1) as cpool,
            tc.tile_pool(name="stage_s", bufs=2) as spool_s,
            tc.tile_pool(name="stage_l", bufs=6) as spool_l,
            tc.tile_pool(name="psum", bufs=8, space=bass.MemorySpace.PSUM) as ppool,
            tc.tile_pool(name="ot", bufs=8) as otpool,
            tc.tile_pool(name="oc", bufs=12) as ocpool,
        ):
            # HAM warm-up: garbage operands, scratch PSUM from the pool
            # (its buffer is recycled by the real blocks afterwards; the
            # WAW dep via the pool orders them behind the warm-ups).
            wg = cpool.tile([128, 128 + NT], FP8, tag="wg")
            nc.gpsimd.memset(wg[:], 0.0)
            psg = ppool.tile([128, NT], F32, tag="ps")
            for i in range(NWARM):
                nc.tensor.matmul(
                    psg[:],
                    wg[:, 0:128],
                    wg[:, 128 : 128 + NT],
                    start=(i == 0),
                    stop=(i == NWARM - 1),
                )

            # weights: [p, (k, co, j, m)]
            w_sb = cpool.tile([128, KTAPS * WCOL], FP8, tag="w")
            sb_sb = cpool.tile([128, 2 * NCO], F32, tag="sb")

            # Padded binarized activation planes, one tile per image.  The
            # two cin-128-chunks (DoubleRow k-subtiles) are interleaved
            # byte-wise as the innermost dim so every matmul rhs window is a
            # tight flat byte range (keeps RAW dep tracking per row-band).
            xp = {}
            for img in range(BPC):
                t = cpool.tile([128, XT, NCI], FP8, tag=f"xp{img}")
                xp[img] = t
                for j in range(NCI):
                    # zero only the borders: top guard+row0, bottom row57+tail
                    # guard, and the two side columns of rows 1..56.
                    nc.gpsimd.memset(t[:, 0 : GUARD + WP, j], 0.0)
                    nc.gpsimd.memset(t[:, GUARD + (HP - 1) * WP : XT, j], 0.0)
                    side = t[:, GUARD + WP : GUARD + WP + H * WP, j].rearrange(
                        "p (h w) -> p h w", w=WP
                    )
                    nc.gpsimd.memset(side[:, :, 0:1], 0.0)
                    nc.gpsimd.memset(side[:, :, WP - 1 : WP], 0.0)

            def plane_view(img, j):
                return xp[img][:, GUARD : GUARD + IMG, j].rearrange(
                    "p (h w) -> p h w", w=WP
                )

            # --- input DMAs, all on the sync queue in explicit FIFO order
            # (one HW DMA engine serves the dynamic queues, so program
            # order on one queue fixes arrival order): img0 chunk0 first,
            # then the 9 per-tap weight slices (first taps gate the first
            # matmuls; later taps only need to stay ~200ns/tap ahead),
            # scale/bias, then img0 chunks 1..3.
            rch0 = CHUNKS[0]
            stc = spool_s.tile([128, NCI, rch0 * W], BF16, tag="stage_c0")
            for j in range(NCI):
                nc.sync.dma_start(
                    stc[:, j, :], xin[0, j * 128 : (j + 1) * 128, 0 : rch0 * W]
                )
            for k in range(KTAPS):
                nc.sync.dma_start(
                    w_sb[:, k * WCOL : (k + 1) * WCOL],
                    wts[:, k * WCOL : (k + 1) * WCOL],
                )
            nc.sync.dma_start(sb_sb[:], sb[:])

            full = xp[0][:, GUARD : GUARD + IMG, :].rearrange(
                "p (h w) j -> p j h w", w=WP
            )
            gate = nc.scalar.sign(
                full[:, :, 1 : 1 + rch0, 1 : W + 1],
                stc.rearrange("p j (h w) -> p j h w", w=W),
            )
            r0 = rch0
            for c, rch in enumerate(CHUNKS[1:], 1):
                for j in range(NCI):
                    st = spool_s.tile([128, rch * W], BF16, tag=f"stage_s{c}")
                    nc.sync.dma_start(
                        st[:],
                        xin[0, j * 128 : (j + 1) * 128, r0 * W : (r0 + rch) * W],
                    )
                    interior = plane_view(0, j)[:, 1 + r0 : 1 + r0 + rch, 1 : W + 1]
                    nc.scalar.sign(interior, st.rearrange("p (h w) -> p h w", w=W))
                r0 += rch

            # Images 1..3: whole-plane loads (fewer DMA descriptors on Sync),
            # held back behind img0's critical chunks so those get full HBM
            # bandwidth and the first matmul starts as early as possible.
            for img in range(1, BPC):
                for j in range(NCI):
                    st = spool_l.tile([128, H * W], BF16, tag="stage_l")
                    dma = nc.gpsimd.dma_start(st[:], xin[img, j * 128 : (j + 1) * 128, :])
                    tile.add_dep_helper(
                        dma.ins,
                        gate.ins,
                        sync=True,
                        reason="stagger bulk input loads behind img0 critical path",
                    )
                    interior = plane_view(img, j)[:, 1 : H + 1, 1 : W + 1]
                    nc.scalar.sign(interior, st.rearrange("p (h w) -> p h w", w=W))

            w4 = w_sb.rearrange("p (k co j m) -> p k co j m", k=KTAPS, co=NCO, j=NCI)
            for img in range(BPC):
                for co in range(NCO):
                    s_ap = sb_sb[:, co : co + 1]
                    b_ap = sb_sb[:, NCO + co : NCO + co + 1]
                    # (start padded row, rows) per output tile; the final tile
                    # of the kernel is split so the last epilogue+store chain
                    # after the last matmul is as short as possible.
                    blocks = [(1 + b * RB, RB) for b in range(NBLK)]
                    if img == BPC - 1 and co == NCO - 1:
                        blocks = blocks[:-1] + [(49, 4), (53, 4)]
                    for y0p, rb in blocks:
                        nt = rb * WP
                        ps = ppool.tile([128, nt], F32, tag="ps")
                        for k in range(KTAPS):
                            ky, kx = divmod(k, 3)
                            s0 = GUARD + (y0p + ky - 1) * WP + (kx - 1)
                            rhs = xp[img][:, s0 : s0 + nt, :].rearrange(
                                "p x j -> p j x"
                            )
                            nc.tensor.matmul(
                                ps[:],
                                w4[:, k, co],
                                rhs,
                                start=(k == 0),
                                stop=(k == KTAPS - 1),
                                perf_mode=mybir.MatmulPerfMode.DoubleRow,
                            )
                        ot = otpool.tile([128, nt], F32, tag="ot")
                        nc.vector.tensor_scalar(
                            ot[:],
                            ps[:],
                            s_ap,
                            b_ap,
                            op0=mybir.AluOpType.mult,
                            op1=mybir.AluOpType.add,
                        )
                        # clip + compact away the 2 garbage cols per row, so
                        # both sides of the output DMA are fully contiguous
                        oc = ocpool.tile([128, rb * W], BF16, tag="oc")
                        nc.vector.tensor_scalar(
                            oc[:],
                            ot.rearrange("p (r c) -> p r c", c=WP)[:, :, 1 : W + 1],
                            -1.0,
                            1.0,
                            op0=mybir.AluOpType.max,
                            op1=mybir.AluOpType.min,
                        )
                        nc.sync.dma_start(
                            yout[img, co * 128 : (co + 1) * 128, y0p - 1 : y0p - 1 + rb, :],
                            oc[:],
                        )
    nc.finalize()
    return nc


def get_nc() -> bass.Bass:
    if "nc" not in _CACHE:
        _CACHE["nc"] = _build_nc()
    return _CACHE["nc"]


def _host_prep(weight, gamma, beta, running_mean, running_var):
    """Binarize standardized weights, fold sw + BN into scale/bias."""
    wf = weight.reshape(COUT, -1).astype(np.float64)
    n = wf.shape[1]
    mean = wf.mean(axis=1, keepdims=True)
    d = wf - mean
    sgn = np.where(d >= 0, 1.0, -1.0)
    std = np.sqrt((d * d).sum(axis=1, keepdims=True) / (n - 1))
    bw = d / std
    sw = np.exp2(np.round(np.log2(np.abs(bw).mean(axis=1))))  # [COUT]
    inv = gamma.astype(np.float64) / np.sqrt(running_var.astype(np.float64) + BN_EPS)
    scale = (sw * inv).astype(np.float32)
    bias = (beta.astype(np.float64) - running_mean.astype(np.float64) * inv).astype(
        np.float32
    )

    # wts[p, (k, co, j, m)] = sgn[co*128+m, (j*128+p)*9 + k]
    fp8np = mybir.dt.np(FP8)
    w6 = sgn.reshape(NCO, 128, NCI, 128, KTAPS)  # [co, m, j, p, k]
    wts = (
        np.ascontiguousarray(np.transpose(w6, (3, 4, 0, 2, 1)))  # p k co j m
        .reshape(128, KTAPS * NCO * NCI * 128)
        .astype(fp8np)
    )
    # sb[m, co] = scale chunk, sb[m, NCO+co] = bias chunk
    sbarr = np.concatenate(
        [scale.reshape(NCO, 128).T, bias.reshape(NCO, 128).T], axis=1
    ).astype(np.float32)
    sbarr = np.ascontiguousarray(sbarr)
    return wts, sbarr


def run(x, weight, gamma, beta, running_mean, running_var, trace=False, **tkw):
    x = np.asarray(x, dtype=np.float32)
    wts, sbarr = _host_prep(
        np.asarray(weight, dtype=np.float32),
        np.asarray(gamma, dtype=np.float32),
        np.asarray(beta, dtype=np.float32),
        np.asarray(running_mean, dtype=np.float32),
        np.asarray(running_var, dtype=np.float32),
    )
    import ml_dtypes

    # bf16 truncation of x preserves every sign bit (min |x| >> bf16 denormal
    # range), and sign() is all the kernel reads from x — halves input DMA.
    xb = np.ascontiguousarray(
        x.reshape(B, CIN, H * W).view(np.uint16)[..., 1::2]
    ).view(ml_dtypes.bfloat16)
    in_maps = [
        {
            "xin": xb[c * BPC : (c + 1) * BPC],
            "wts": wts,
            "sb": sbarr,
        }
        for c in range(NCORES)
    ]
    nc = get_nc()
    res = run_bass_kernel_spmd(nc, in_maps, list(range(NCORES)), trace=trace, **tkw)
    y = np.concatenate([r["yout"] for r in res.results], axis=0)
    return y.astype(np.float32), res


def kernel(x, weight, gamma, beta, running_mean, running_var):
    y, _ = run(x, weight, gamma, beta, running_mean, running_var)
    return y


# revision 3
# speedup vs baseline: 1.1140x; 1.1140x over previous
"""Trainium2 Bass kernel for IR-Net style binarized 3x3 conv + BN + Hardtanh.

Reference computation:
  bw = sign(standardize(weight)) * sw   (sw = per-cout power-of-2 scale)
  ba = sign(x)
  y  = clip(conv3x3(ba, bw) * bn_scale + bn_bias, -1, 1)

Both matmul operands are exactly +-1, which is exactly representable in
fp8e4m3, so the conv runs as fp8 DoubleRow matmuls on the TensorEngine
with zero numerical error (fp32 PSUM accumulation of integers <= 2304).
Weight standardization/sign, sw, and BN folding are host-side prep
(0.6 MB of data); sw and bn scale fold into a single per-channel scale
applied in the epilogue (on VectorE, so ScalarE is free for binarize).

Distribution: pure data parallel, 32 images -> 4 per NeuronCore, full
weights replicated, no collectives.

Layout: per-image zero-padded activation planes in SBUF, fp8, rows of
57 = [left pad, x0..x55]: the NEXT row's left pad doubles as this row's
right pad, so a row stride of 57 (not 58) suffices and each conv tap is
a contiguous shifted window of the flattened plane.  The two cin-128-
chunks are byte-interleaved as the DoubleRow k-subtile dim, so the conv
is 9 accumulated DoubleRow matmuls ([128,2,128] @ [128,2,456], K=256)
per 8-row output tile.  One garbage column per row (plus the window
tail) is discarded by the epilogue compact.

Timing model (v1 measured 121.5us): the warm matmul stream runs at the
exact N-cycle/2.4GHz floor, so everything else is edge overhead:
 - 9 warm-up matmuls on memset garbage issue as the first Tensor
   instructions; the PE is otherwise idle during the DMA ramp and ~3.5us
   of sustained PE activity flips the HAM clock gate to 8/8 early, so
   the real stream runs warm almost from its first matmul (v1 paid 14
   matmuls at the 1.2GHz cold clock and started 2.7us later).
 - All critical input DMAs ride the sync queue in explicit FIFO order
   (the dynamic queues share one HW DMA engine, so cross-queue arrival
   order is not controllable): img0 chunk0, all weights, then img0
   chunks 1..6, sized so every sign() lands >=1us before the matmul
   block that consumes it.  Bulk img1-3 loads go on the gpsimd queue
   gated behind the last img0 sign.
 - Output is stored as bf16 (exact for the clipped +-1 majority, 2^-9
   relative on the rest) halving output DMA; host casts back to fp32.
"""

import numpy as np

import concourse.bass as bass
import concourse.bacc as bacc
import concourse.mybir as mybir
import concourse.tile as tile
from concourse.bass_utils import run_bass_kernel_spmd

B, CIN, COUT, H, W = 32, 256, 256, 56, 56
NCORES = 8
BPC = B // NCORES            # images per core
HP, WP = H + 2, W + 1        # 58 padded rows of [pad, x0..x55]
IMG = HP * WP                # 3306
GUARD = 64                   # front zero guard (shifted windows stay in bounds)
XT = 3376                    # GUARD + IMG + tail guard(6); stride*NCI %16==0
RB = 8                       # output rows per tile
NBLK = H // RB               # 7
NT = RB * WP                 # 456 matmul free dim (incl. 1 garbage col/row)
NCI = CIN // 128             # 2 cin chunks = DoubleRow k-subtiles
NCO = COUT // 128            # 2 cout chunks
KTAPS = 9
NWARM = 9                    # HAM warm-up matmuls (~3.5us cold PE activity)
BN_EPS = 1e-5

F32 = mybir.dt.float32
FP8 = mybir.dt.float8e4
BF16 = mybir.dt.bfloat16

_CACHE: dict = {}


def _build_nc() -> bass.Bass:
    nc = bacc.Bacc("TRN2", target_bir_lowering=False, debug=False, num_devices=NCORES)
    xin = nc.declare_dram_parameter("xin", [BPC, CIN, H * W], BF16, isOutput=False)
    wts = nc.declare_dram_parameter(
        "wts", [128, KTAPS * NCO * NCI * 128], FP8, isOutput=False
    )
    sb = nc.declare_dram_parameter("sb", [128, 2 * NCO], F32, isOutput=False)
    yout = nc.declare_dram_parameter("yout", [BPC, COUT, H, W], BF16, isOutput=True)

    # img0 binarize chunks: chunk c covers x rows [cum, cum+rows); block b
    # (output rows 8b..8b+7) needs x rows <= 8b+8, each sign lands >~1us
    # before its consumer given the FIFO DMA schedule below.
    CHUNKS = [10, 7, 8, 8, 8, 8, 7]
    assert sum(CHUNKS) == H

    with tile.TileContext(nc) as tc:
        with (
            tc.tile_pool(name="const", bufs=1) as cpool,
            tc.tile_pool(name="stage_s", bufs=3) as spool_s,
            tc.tile_pool(name="stage_l", bufs=6) as spool_l,
            tc.tile_pool(name="psum", bufs=8, space=bass.MemorySpace.PSUM) as ppool,
            tc.tile_pool(name="ot", bufs=8) as otpool,
            tc.tile_pool(name="oc", bufs=12) as ocpool,
        ):
            # HAM warm-up: garbage operands, scratch PSUM from the pool
            # (its buffer is recycled by the real blocks afterwards; the
            # WAW dep via the pool orders them behind the warm-ups).
            wg = cpool.tile([128, 128 + NT], FP8, tag="wg")
            nc.gpsimd.memset(wg[:], 0.0)
            psg = ppool.tile([128, NT], F32, tag="ps")
            for i in range(NWARM):
                nc.tensor.matmul(
                    psg[:],
                    wg[:, 0:128],
                    wg[:, 128 : 128 + NT],
                    start=(i == 0),
                    stop=(i == NWARM - 1),
                )

            w_sb = cpool.tile([128, KTAPS * NCO * NCI * 128], FP8, tag="w")
            sb_sb = cpool.tile([128, 2 * NCO], F32, tag="sb")

            # Padded binarized activation planes, one tile per image.  The
            # two cin-128-chunks (DoubleRow k-subtiles) are interleaved
            # byte-wise as the innermost dim so every matmul rhs window is a
            # tight flat byte range (keeps RAW dep tracking per row-band).
            xp = {}
            for img in range(BPC):
                t = cpool.tile([128, XT, NCI], FP8, tag=f"xp{img}")
                xp[img] = t
                for j in range(NCI):
                    # borders: front guard + top pad row, bottom pad row +
                    # tail guard, and the left pad column of rows 1..57
                    # (the left pad of row r+1 is also row r's right pad).
                    nc.gpsimd.memset(t[:, 0 : GUARD + WP, j], 0.0)
                    nc.gpsimd.memset(t[:, GUARD + (HP - 1) * WP : XT, j], 0.0)
                    side = t[:, GUARD + WP : GUARD + WP + (HP - 2) * WP, j].rearrange(
                        "p (h w) -> p h w", w=WP
                    )
                    nc.gpsimd.memset(side[:, :, 0:1], 0.0)

            def interior(img, r0, rch):
                # [p, j, rch rows, 56 cols] view of x rows r0..r0+rch
                return (
                    xp[img][:, GUARD : GUARD + IMG, :]
                    .rearrange("p (h w) j -> p j h w", w=WP)[
                        :, :, 1 + r0 : 1 + r0 + rch, 1:WP
                    ]
                )

            # --- critical input DMAs in explicit FIFO order on the sync
            # queue: img0 chunk0, weights, chunk1, scale/bias, chunks 2..6.
            def chunk_dma(img, r0, rch, tag):
                st = spool_s.tile([128, NCI, rch * W], BF16, tag=tag)
                src = xin[img, :, r0 * W : (r0 + rch) * W].rearrange(
                    "(j p) x -> p j x", p=128
                )
                nc.sync.dma_start(st[:], src)
                return st

            st0 = chunk_dma(0, 0, CHUNKS[0], "stage_c0")
            nc.sync.dma_start(w_sb[:], wts[:])
            st1 = chunk_dma(0, CHUNKS[0], CHUNKS[1], "stage_c1")
            nc.sync.dma_start(sb_sb[:], sb[:])
            stages = [(0, CHUNKS[0], st0), (CHUNKS[0], CHUNKS[1], st1)]
            r0 = CHUNKS[0] + CHUNKS[1]
            for c, rch in enumerate(CHUNKS[2:], 2):
                stages.append((r0, rch, chunk_dma(0, r0, rch, f"stage_c{c}")))
                r0 += rch

            gate = None
            for r0, rch, st in stages:
                gate = nc.scalar.sign(
                    interior(0, r0, rch),
                    st.rearrange("p j (h w) -> p j h w", w=W),
                )

            # Images 1..3: whole-plane loads on the gpsimd queue, held back
            # behind img0's last sign so they never contend with the
            # critical img0 chunks + weights on the shared DMA engine.
            for img in range(1, BPC):
                for j in range(NCI):
                    st = spool_l.tile([128, H * W], BF16, tag="stage_l")
                    dma = nc.gpsimd.dma_start(st[:], xin[img, j * 128 : (j + 1) * 128, :])
                    tile.add_dep_helper(
                        dma.ins,
                        gate.ins,
                        sync=True,
                        reason="stagger bulk input loads behind img0 critical path",
                    )
                    dst = interior(img, 0, H)[:, j]
                    nc.scalar.sign(dst, st.rearrange("p (h w) -> p h w", w=W))

            w4 = w_sb.rearrange("p (k co j m) -> p k co j m", k=KTAPS, co=NCO, j=NCI)
            for img in range(BPC):
                for co in range(NCO):
                    s_ap = sb_sb[:, co : co + 1]
                    b_ap = sb_sb[:, NCO + co : NCO + co + 1]
                    # (first output row, rows) per tile; the final tile of
                    # the kernel is split so the last epilogue+store chain
                    # after the last matmul is as short as possible.
                    blocks = [(b * RB, RB) for b in range(NBLK)]
                    if img == BPC - 1 and co == NCO - 1:
                        blocks = blocks[:-1] + [(48, 4), (52, 4)]
                    for orow, rb in blocks:
                        nt = rb * WP
                        ps = ppool.tile([128, nt], F32, tag="ps")
                        for k in range(KTAPS):
                            ky, kx = divmod(k, 3)
                            s0 = GUARD + (orow + ky) * WP + (kx - 1)
                            rhs = xp[img][:, s0 : s0 + nt, :].rearrange(
                                "p x j -> p j x"
                            )
                            nc.tensor.matmul(
                                ps[:],
                                w4[:, k, co],
                                rhs,
                                start=(k == 0),
                                stop=(k == KTAPS - 1),
                                perf_mode=mybir.MatmulPerfMode.DoubleRow,
                            )
                        ot = otpool.tile([128, nt], F32, tag="ot")
                        nc.vector.tensor_scalar(
                            ot[:],
                            ps[:],
                            s_ap,
                            b_ap,
                            op0=mybir.AluOpType.mult,
                            op1=mybir.AluOpType.add,
                        )
                        # clip + compact away the garbage col per row, so
                        # both sides of the output DMA are fully contiguous
                        oc = ocpool.tile([128, rb * W], BF16, tag="oc")
                        nc.vector.tensor_scalar(
                            oc[:],
                            ot.rearrange("p (r c) -> p r c", c=WP)[:, :, 1:WP],
                            -1.0,
                            1.0,
                            op0=mybir.AluOpType.max,
                            op1=mybir.AluOpType.min,
                        )
                        nc.sync.dma_start(
                            yout[img, co * 128 : (co + 1) * 128, orow : orow + rb, :],
                            oc[:],
                        )
    nc.finalize()
    return nc


def get_nc() -> bass.Bass:
    if "nc" not in _CACHE:
        _CACHE["nc"] = _build_nc()
    return _CACHE["nc"]


def _host_prep(weight, gamma, beta, running_mean, running_var):
    """Binarize standardized weights, fold sw + BN into scale/bias."""
    wf = weight.reshape(COUT, -1).astype(np.float64)
    n = wf.shape[1]
    mean = wf.mean(axis=1, keepdims=True)
    d = wf - mean
    sgn = np.where(d >= 0, 1.0, -1.0)
    std = np.sqrt((d * d).sum(axis=1, keepdims=True) / (n - 1))
    bw = d / std
    sw = np.exp2(np.round(np.log2(np.abs(bw).mean(axis=1))))  # [COUT]
    inv = gamma.astype(np.float64) / np.sqrt(running_var.astype(np.float64) + BN_EPS)
    scale = (sw * inv).astype(np.float32)
    bias = (beta.astype(np.float64) - running_mean.astype(np.float64) * inv).astype(
        np.float32
    )

    # wts[p, (k, co, j, m)] = sgn[co*128+m, (j*128+p)*9 + k]
    fp8np = mybir.dt.np(FP8)
    w6 = sgn.reshape(NCO, 128, NCI, 128, KTAPS)  # [co, m, j, p, k]
    wts = (
        np.ascontiguousarray(np.transpose(w6, (3, 4, 0, 2, 1)))  # p k co j m
        .reshape(128, KTAPS * NCO * NCI * 128)
        .astype(fp8np)
    )
    # sb[m, co] = scale chunk, sb[m, NCO+co] = bias chunk
    sbarr = np.concatenate(
        [scale.reshape(NCO, 128).T, bias.reshape(NCO, 128).T], axis=1
    ).astype(np.float32)
    sbarr = np.ascontiguousarray(sbarr)
    return wts, sbarr


def run(x, weight, gamma, beta, running_mean, running_var, trace=False, **tkw):
    x = np.asarray(x, dtype=np.float32)
    wts, sbarr = _host_prep(
        np.asarray(weight, dtype=np.float32),
        np.asarray(gamma, dtype=np.float32),
        np.asarray(beta, dtype=np.float32),
        np.asarray(running_mean, dtype=np.float32),
        np.asarray(running_var, dtype=np.float32),
    )
    import ml_dtypes

    # bf16 truncation of x preserves every sign bit (min |x| >> bf16 denormal
    # range), and sign() is all the kernel reads from x — halves input DMA.
    xb = np.ascontiguousarray(
        x.reshape(B, CIN, H * W).view(np.uint16)[..., 1::2]
    ).view(ml_dtypes.bfloat16)
    in_maps = [
        {
            "xin": xb[c * BPC : (c + 1) * BPC],
            "wts": wts,
            "sb": sbarr,
        }
        for c in range(NCORES)
    ]
    nc = get_nc()
    res = run_bass_kernel_spmd(nc, in_maps, list(range(NCORES)), trace=trace, **tkw)
    y = np.concatenate([r["yout"] for r in res.results], axis=0)
    return y.astype(np.float32), res


def kernel(x, weight, gamma, beta, running_mean, running_var):
    y, _ = run(x, weight, gamma, beta, running_mean, running_var)
    return y


# revision 7
# speedup vs baseline: 1.1549x; 1.0367x over previous
"""Trainium2 Bass kernel for IR-Net style binarized 3x3 conv + BN + Hardtanh.

Reference computation:
  bw = sign(standardize(weight)) * sw   (sw = per-cout power-of-2 scale)
  ba = sign(x)
  y  = clip(conv3x3(ba, bw) * bn_scale + bn_bias, -1, 1)

Both matmul operands are exactly +-1, which is exactly representable in
fp8e4m3, so the conv runs as fp8 DoubleRow matmuls on the TensorEngine
with zero numerical error (fp32 PSUM accumulation of integers <= 2304).
Weight standardization/sign, sw, and BN folding are host-side prep
(0.6 MB of data); sw and bn scale fold into a single per-channel scale
applied in the epilogue (on VectorE, so ScalarE is free for binarize).

Distribution: pure data parallel, 32 images -> 4 per NeuronCore, full
weights replicated, no collectives.

Layout: per-image zero-padded activation planes in SBUF, fp8, rows of
57 = [left pad, x0..x55]: the NEXT row's left pad doubles as this row's
right pad, so a row stride of 57 (not 58) suffices and each conv tap is
a contiguous shifted window of the flattened plane.  The two cin-128-
chunks are byte-interleaved as the DoubleRow k-subtile dim, so the conv
is 9 accumulated DoubleRow matmuls ([128,2,128] @ [128,2,456], K=256)
per 8-row output tile.  One garbage column per row (plus the window
tail) is discarded by the epilogue compact.

Timing model (v1 measured 121.5us): the warm matmul stream runs at the
exact N-cycle/2.4GHz floor, so everything else is edge overhead:
 - 9 warm-up matmuls on memset garbage issue as the first Tensor
   instructions; the PE is otherwise idle during the DMA ramp and ~3.5us
   of sustained PE activity flips the HAM clock gate to 8/8 early, so
   the real stream runs warm almost from its first matmul (v1 paid 14
   matmuls at the 1.2GHz cold clock and started 2.7us later).
 - All critical input DMAs ride the sync queue in explicit FIFO order
   (the dynamic queues share one HW DMA engine, so cross-queue arrival
   order is not controllable): img0 chunk0, all weights, then img0
   chunks 1..6, sized so every sign() lands >=1us before the matmul
   block that consumes it.  Bulk img1-3 loads go on the gpsimd queue
   gated behind the last img0 sign.
 - Output is stored as bf16 (exact for the clipped +-1 majority, 2^-9
   relative on the rest) halving output DMA; host casts back to fp32.
"""

import numpy as np

import concourse.bass as bass
import concourse.bacc as bacc
import concourse.mybir as mybir
import concourse.tile as tile
from concourse.bass_utils import run_bass_kernel_spmd

B, CIN, COUT, H, W = 32, 256, 256, 56, 56
NCORES = 8
BPC = B // NCORES            # images per core
HP, WP = H + 2, W + 1        # 58 padded rows of [pad, x0..x55]
IMG = HP * WP                # 3306
GUARD = 64                   # front zero guard (shifted windows stay in bounds)
XT = 3376                    # GUARD + IMG + tail guard(6); stride*NCI %16==0
RB = 8                       # output rows per tile
NBLK = H // RB               # 7
NT = RB * WP                 # 456 matmul free dim (incl. 1 garbage col/row)
NCI = CIN // 128             # 2 cin chunks = DoubleRow k-subtiles
NCO = COUT // 128            # 2 cout chunks
KTAPS = 9
NWARM_LONG = 9               # HAM warm-up: 9 full-width matmuls (~3.4us cold)
NWARM_SHORT = 10             # + short ones bridging until real data arrives
NTS = 114                    # short warm-up free dim (~107ns each cold)
BN_EPS = 1e-5

F32 = mybir.dt.float32
FP8 = mybir.dt.float8e4
BF16 = mybir.dt.bfloat16

_CACHE: dict = {}


def _build_nc() -> bass.Bass:
    nc = bacc.Bacc("TRN2", target_bir_lowering=False, debug=False, num_devices=NCORES)
    xin = nc.declare_dram_parameter("xin", [BPC, CIN, H * W], BF16, isOutput=False)
    wts = nc.declare_dram_parameter(
        "wts", [128, KTAPS * NCO * NCI * 128], FP8, isOutput=False
    )
    sb = nc.declare_dram_parameter("sb", [128, 2 * NCO], F32, isOutput=False)
    yout = nc.declare_dram_parameter("yout", [BPC, COUT, H, W], BF16, isOutput=True)

    # img0 binarize chunks: chunk c covers x rows [cum, cum+rows); block b
    # (output rows 8b..8b+7) needs x rows <= 8b+8, each sign lands >~1us
    # before its consumer given the FIFO DMA schedule below.
    CHUNKS = [10, 7, 8, 8, 8, 8, 7]
    assert sum(CHUNKS) == H

    with tile.TileContext(nc) as tc:
        with (
            tc.tile_pool(name="const", bufs=1) as cpool,
            tc.tile_pool(name="stage_s", bufs=3) as spool_s,
            tc.tile_pool(name="stage_l", bufs=6) as spool_l,
            tc.tile_pool(name="psum", bufs=8, space=bass.MemorySpace.PSUM) as ppool,
            tc.tile_pool(name="ot", bufs=8) as otpool,
            tc.tile_pool(name="oc", bufs=12) as ocpool,
        ):
            # HAM warm-up: garbage operands, scratch PSUM from the pool
            # (its buffer is recycled by the real blocks afterwards; the
            # WAW dep via the pool orders them behind the warm-ups).  PE
            # activity must be CONTINUOUS from here into the real stream
            # (any idle gap restarts the 3.4us HAM busy window), so after
            # ~3.4us of full-width matmuls a run of short ones bridges the
            # remaining time until the first real operands have landed.
            wg = cpool.tile([128, 128 + NT], FP8, tag="wg")
            nc.vector.memset(wg[:], 0.0)
            psg = ppool.tile([128, NT], F32, tag="ps")
            nwarm = NWARM_LONG + NWARM_SHORT
            for i in range(nwarm):
                n = NT if i < NWARM_LONG else NTS
                nc.tensor.matmul(
                    psg[:, 0:n],
                    wg[:, 0:128],
                    wg[:, 128 : 128 + n],
                    start=(i == 0),
                    stop=(i == nwarm - 1),
                )

            w_sb = cpool.tile([128, KTAPS * NCO * NCI * 128], FP8, tag="w")
            sb_sb = cpool.tile([128, 2 * NCO], F32, tag="sb")

            # Padded binarized activation planes, one tile per image.  The
            # two cin-128-chunks (DoubleRow k-subtiles) are interleaved
            # byte-wise as the innermost dim so every matmul rhs window is a
            # tight flat byte range (keeps RAW dep tracking per row-band).
            xp = {}
            for img in range(BPC):
                t = cpool.tile([128, XT, NCI], FP8, tag=f"xp{img}")
                xp[img] = t
                for j in range(NCI):
                    # borders: front guard + top pad row, bottom pad row +
                    # tail guard, and the left pad column of rows 1..57
                    # (the left pad of row r+1 is also row r's right pad).
                    nc.gpsimd.memset(t[:, 0 : GUARD + WP, j], 0.0)
                    nc.gpsimd.memset(t[:, GUARD + (HP - 1) * WP : XT, j], 0.0)
                    side = t[:, GUARD + WP : GUARD + WP + (HP - 2) * WP, j].rearrange(
                        "p (h w) -> p h w", w=WP
                    )
                    nc.gpsimd.memset(side[:, :, 0:1], 0.0)

            def interior(img, r0, rch):
                # [p, j, rch rows, 56 cols] view of x rows r0..r0+rch
                return (
                    xp[img][:, GUARD : GUARD + IMG, :]
                    .rearrange("p (h w) j -> p j h w", w=WP)[
                        :, :, 1 + r0 : 1 + r0 + rch, 1:WP
                    ]
                )

            # --- critical input DMAs in explicit FIFO order on the sync
            # queue: img0 chunks 0-1, weights (split so the first matmuls
            # gate on the taps 0-4 slice only), scale/bias, chunks 2..6.
            def chunk_dma(img, r0, rch, tag):
                st = spool_s.tile([128, NCI, rch * W], BF16, tag=tag)
                src = xin[img, :, r0 * W : (r0 + rch) * W].rearrange(
                    "(j p) x -> p j x", p=128
                )
                nc.sync.dma_start(st[:], src)
                return st

            WCOL = NCO * NCI * 128
            st0 = chunk_dma(0, 0, CHUNKS[0], "stage_c0")
            st1 = chunk_dma(0, CHUNKS[0], CHUNKS[1], "stage_c1")
            nc.sync.dma_start(w_sb[:, : 5 * WCOL], wts[:, : 5 * WCOL])
            nc.sync.dma_start(w_sb[:, 5 * WCOL :], wts[:, 5 * WCOL :])
            nc.sync.dma_start(sb_sb[:], sb[:])
            stages = [(0, CHUNKS[0], st0), (CHUNKS[0], CHUNKS[1], st1)]
            r0 = CHUNKS[0] + CHUNKS[1]
            for c, rch in enumerate(CHUNKS[2:], 2):
                stages.append((r0, rch, chunk_dma(0, r0, rch, f"stage_c{c}")))
                r0 += rch

            gate = None
            for r0, rch, st in stages:
                gate = nc.scalar.sign(
                    interior(0, r0, rch),
                    st.rearrange("p j (h w) -> p j h w", w=W),
                )

            # Images 1..3: whole-plane loads on the gpsimd queue, held back
            # behind img0's last sign so they never contend with the
            # critical img0 chunks + weights on the shared DMA engine.
            for img in range(1, BPC):
                for j in range(NCI):
                    st = spool_l.tile([128, H * W], BF16, tag="stage_l")
                    dma = nc.gpsimd.dma_start(st[:], xin[img, j * 128 : (j + 1) * 128, :])
                    tile.add_dep_helper(
                        dma.ins,
                        gate.ins,
                        sync=True,
                        reason="stagger bulk input loads behind img0 critical path",
                    )
                    dst = interior(img, 0, H)[:, j]
                    nc.scalar.sign(dst, st.rearrange("p (h w) -> p h w", w=W))

            w4 = w_sb.rearrange("p (k co j m) -> p k co j m", k=KTAPS, co=NCO, j=NCI)
            for img in range(BPC):
                for co in range(NCO):
                    s_ap = sb_sb[:, co : co + 1]
                    b_ap = sb_sb[:, NCO + co : NCO + co + 1]
                    # (first output row, rows) per tile; the final tile of
                    # the kernel is split so the last epilogue+store chain
                    # after the last matmul is as short as possible.
                    blocks = [(b * RB, RB) for b in range(NBLK)]
                    if img == BPC - 1 and co == NCO - 1:
                        blocks = blocks[:-1] + [(48, 4), (52, 4)]
                    for orow, rb in blocks:
                        nt = rb * WP
                        ps = ppool.tile([128, nt], F32, tag="ps")
                        for k in range(KTAPS):
                            ky, kx = divmod(k, 3)
                            s0 = GUARD + (orow + ky) * WP + (kx - 1)
                            # Border taps read a whole zero-pad row (ky=0 at
                            # the top edge / ky=2 at the bottom): shorten the
                            # window by one row — the untouched PSUM range is
                            # simply overwritten by the next full tap via
                            # has_written.  The start MM stays full-width so
                            # the accumulation group covers the whole bank.
                            p0, pn = 0, nt
                            if k != 0:
                                if orow == 0 and ky == 0:
                                    p0 = WP
                                elif orow + rb == H and ky == 2:
                                    pn = nt - WP
                            rhs = xp[img][:, s0 + p0 : s0 + pn, :].rearrange(
                                "p x j -> p j x"
                            )
                            nc.tensor.matmul(
                                ps[:, p0:pn],
                                w4[:, k, co],
                                rhs,
                                start=(k == 0),
                                stop=(k == KTAPS - 1),
                                perf_mode=mybir.MatmulPerfMode.DoubleRow,
                            )
                        ot = otpool.tile([128, nt], F32, tag="ot")
                        nc.vector.tensor_scalar(
                            ot[:],
                            ps[:],
                            s_ap,
                            b_ap,
                            op0=mybir.AluOpType.mult,
                            op1=mybir.AluOpType.add,
                        )
                        # clip + compact away the garbage col per row, so
                        # both sides of the output DMA are fully contiguous
                        oc = ocpool.tile([128, rb * W], BF16, tag="oc")
                        nc.vector.tensor_scalar(
                            oc[:],
                            ot.rearrange("p (r c) -> p r c", c=WP)[:, :, 1:WP],
                            -1.0,
                            1.0,
                            op0=mybir.AluOpType.max,
                            op1=mybir.AluOpType.min,
                        )
                        nc.sync.dma_start(
                            yout[img, co * 128 : (co + 1) * 128, orow : orow + rb, :],
                            oc[:],
                        )
    nc.finalize()
    return nc


def get_nc() -> bass.Bass:
    if "nc" not in _CACHE:
        _CACHE["nc"] = _build_nc()
    return _CACHE["nc"]


def _host_prep(weight, gamma, beta, running_mean, running_var):
    """Binarize standardized weights, fold sw + BN into scale/bias."""
    wf = weight.reshape(COUT, -1).astype(np.float64)
    n = wf.shape[1]
    mean = wf.mean(axis=1, keepdims=True)
    d = wf - mean
    sgn = np.where(d >= 0, 1.0, -1.0)
    std = np.sqrt((d * d).sum(axis=1, keepdims=True) / (n - 1))
    bw = d / std
    sw = np.exp2(np.round(np.log2(np.abs(bw).mean(axis=1))))  # [COUT]
    inv = gamma.astype(np.float64) / np.sqrt(running_var.astype(np.float64) + BN_EPS)
    scale = (sw * inv).astype(np.float32)
    bias = (beta.astype(np.float64) - running_mean.astype(np.float64) * inv).astype(
        np.float32
    )

    # wts[p, (k, co, j, m)] = sgn[co*128+m, (j*128+p)*9 + k]
    fp8np = mybir.dt.np(FP8)
    w6 = sgn.reshape(NCO, 128, NCI, 128, KTAPS)  # [co, m, j, p, k]
    wts = (
        np.ascontiguousarray(np.transpose(w6, (3, 4, 0, 2, 1)))  # p k co j m
        .reshape(128, KTAPS * NCO * NCI * 128)
        .astype(fp8np)
    )
    # sb[m, co] = scale chunk, sb[m, NCO+co] = bias chunk
    sbarr = np.concatenate(
        [scale.reshape(NCO, 128).T, bias.reshape(NCO, 128).T], axis=1
    ).astype(np.float32)
    sbarr = np.ascontiguousarray(sbarr)
    return wts, sbarr


def run(x, weight, gamma, beta, running_mean, running_var, trace=False, **tkw):
    x = np.asarray(x, dtype=np.float32)
    wts, sbarr = _host_prep(
        np.asarray(weight, dtype=np.float32),
        np.asarray(gamma, dtype=np.float32),
        np.asarray(beta, dtype=np.float32),
        np.asarray(running_mean, dtype=np.float32),
        np.asarray(running_var, dtype=np.float32),
    )
    import ml_dtypes

    # bf16 truncation of x preserves every sign bit (min |x| >> bf16 denormal
    # range), and sign() is all the kernel reads from x — halves input DMA.
    xb = np.ascontiguousarray(
        x.reshape(B, CIN, H * W).view(np.uint16)[..., 1::2]
    ).view(ml_dtypes.bfloat16)
    in_maps = [
        {
            "xin": xb[c * BPC : (c + 1) * BPC],
            "wts": wts,
            "sb": sbarr,
        }
        for c in range(NCORES)
    ]
    nc = get_nc()
    res = run_bass_kernel_spmd(nc, in_maps, list(range(NCORES)), trace=trace, **tkw)
    y = np.concatenate([r["yout"] for r in res.results], axis=0)
    return y.astype(np.float32), res


def kernel(x, weight, gamma, beta, running_mean, running_var):
    y, _ = run(x, weight, gamma, beta, running_mean, running_var)
    return y
